# revision 16
# baseline (speedup 1.0000x reference)
"""CTC loss (keras ctc_batch_cost semantics) on 8 Trainium2 NeuronCores.

Data-parallel over batch: 1024 samples -> 8 cores x 128 samples
(one sample per SBUF partition).  Host prep is integer-only (gather
index tables + skip masks); all float work runs on device.

Device pipeline (per core, per 128-step T-half):
  A. load y_pred [tau,c] sample tiles, ACT-cast fp32->bf16 with +EPS,
     DMA-xbar transpose to [c,tau], store rows to an HBM scratch.
  B. dma_gather: rows (b, c=label_j) and (b, blank) -> ptil[b, (blk, tau)]
     in one shot (per-sample label gathers via int16 row indices).
  C. per-column scale: g = max over a 9-block subset, ginv = e^rhat/g
     (fp32 reciprocal, bf16); ptil *= ginv (GPSIMD).
  D. s-sweep over the 129-row extended CTC lattice: each row's
     recursion v_t = (e_t + v_{t-1}) * p_t is ONE tensor_tensor_scan
     along the free dim; row coupling e_t = v^{s-1}_{t-1} + m*v^{s-2}_{t-1}
     is one scalar_tensor_tensor (odd rows) or a shifted view (even).
     Probability domain, per-half max-renorm keeps fp32 range.
Host assembles loss = -(log lsum + sum log bmax - sum log ginv) in f64.
"""
from contextlib import ExitStack

import numpy as np
import ml_dtypes

import concourse.bass as bass
import concourse.tile as tile
from concourse import bacc, mybir
from concourse.bass_utils import run_bass_kernel_spmd

F32 = mybir.dt.float32
BF16 = mybir.dt.bfloat16
I16 = mybir.dt.int16
AF = mybir.ActivationFunctionType
ALU = mybir.AluOpType

B, T, C, L = 1024, 256, 128, 64
S = 2 * L + 1          # 129 extended states
NBLK = L + 1           # 64 label blocks + 1 blank block
BLANK = C - 1
EPS = 1e-7
RHAT = 0.4             # per-step prob boost exp(RHAT) centers chunk decay
TC = 128               # scan chunk length == tau-half
NCH = T // TC          # 2
W = T + 1              # Treg slot width: col0 = v_{-1}, col 1+t = v_t
SLOTS = S + 2          # 2 permanent zero rows + 129 state rows
PB = 128               # samples per core
NCORES = 8
SGRP = 4               # samples per load/cast group


def _host_prep(y_true_shard: np.ndarray):
    yt = y_true_shard.astype(np.int64)
    idx_flat = np.empty(NBLK * PB, np.int32)
    barange = np.arange(PB) * C
    for j in range(L):
        idx_flat[j * PB:(j + 1) * PB] = barange + yt[:, j]
    idx_flat[L * PB:] = barange + BLANK
    table16 = idx_flat.reshape(NBLK * PB // 16, 16).T      # [16, 520]
    idxs = np.tile(table16, (8, 1)).astype(np.int16)        # [128, 520]
    m01 = np.ones((PB, L), np.float32)
    m01[:, 1:] = (yt[:, 1:] != yt[:, :-1]).astype(np.float32)
    m01[:, 0] = 0.0
    return {"idxs": idxs, "m01": m01}


def _emit(ctx: ExitStack, tc: tile.TileContext, y_in, idxs_in, m01_in,
          raw_out, ginv_out):
    nc = tc.nc

    persist = ctx.enter_context(tc.tile_pool(name="persist", bufs=1))
    stage = ctx.enter_context(tc.tile_pool(name="stage", bufs=4))
    trp = ctx.enter_context(tc.tile_pool(name="trp", bufs=8))
    cpool = ctx.enter_context(tc.tile_pool(name="cbuf", bufs=4))
    scratch = ctx.enter_context(tc.tile_pool(name="scratch", bufs=2))
    dram = ctx.enter_context(tc.tile_pool(name="dram", bufs=1, space="DRAM"))

    idxs = persist.tile([PB, NBLK * PB // 16], I16)
    nc.sync.dma_start(idxs[:], idxs_in[:])
    m01 = persist.tile([PB, L], F32)
    nc.sync.dma_start(m01[:], m01_in[:])

    treg_t = persist.tile([PB, SLOTS * W], F32)
    nc.gpsimd.memset(treg_t[:], 0.0)
    raw = persist.tile([PB, NCH], F32)
    epsb = persist.tile([PB, 1], F32)
    nc.vector.memset(epsb[:], EPS)

    ytT, ptil, ginvb = [], [], []
    for h in range(NCH):
        ytT_h = dram.tile([PB * C, TC], BF16, tag=f"ytT{h}")
        ptil_h = persist.tile([PB, NBLK * TC], BF16, tag=f"ptil{h}")
        ginvb_h = persist.tile([PB, TC], BF16, tag=f"ginvb{h}")
        ytT.append(ytT_h); ptil.append(ptil_h); ginvb.append(ginvb_h)

    def phase_abc(h):
        # A: load + cast(+eps) + xbar transpose + store rows to HBM
        for g in range(PB // SGRP):
            b0 = g * SGRP
            ld = stage.tile([PB, SGRP * C], F32, tag="ld")
            nc.sync.dma_start(
                ld[:].rearrange("p (b c) -> p b c", b=SGRP),
                y_in[b0:b0 + SGRP, h * TC:(h + 1) * TC, :]
                .rearrange("b t c -> t b c"))
            bf = stage.tile([PB, SGRP * C], BF16, tag="bf")
            nc.scalar.activation(bf[:], ld[:], AF.Identity, bias=epsb[:, 0:1])
            for i in range(SGRP):
                b = b0 + i
                tr = trp.tile([C, TC], BF16, tag="tr")
                eng = nc.sync if (i % 2 == 0) else nc.scalar
                eng.dma_start_transpose(tr[:], bf[:, i * C:(i + 1) * C])
                eng2 = nc.scalar if (i % 2 == 0) else nc.sync
                eng2.dma_start(ytT[h][b * C:(b + 1) * C, :], tr[:])
        # B: gathers (8 label-block groups + blank) across SWDGE queues
        for q in range(8):
            nc.gpsimd.dma_gather(
                ptil[h][:, q * 8 * TC:(q + 1) * 8 * TC]
                .rearrange("p (i e) -> p i e", e=TC),
                ytT[h][:],
                idxs[:, 64 * q:64 * q + 64],
                num_idxs=8 * PB, num_idxs_reg=8 * PB,
                elem_size=TC, queue_num=q % 4)
        nc.gpsimd.dma_gather(
            ptil[h][:, L * TC:NBLK * TC]
            .rearrange("p (i e) -> p i e", e=TC),
            ytT[h][:],
            idxs[:, 512:520],
            num_idxs=PB, num_idxs_reg=PB,
            elem_size=TC, queue_num=0)
        # C: per-column scale
        p3 = ptil[h][:].rearrange("p (blk t) -> p t blk", blk=NBLK)
        gmax = scratch.tile([PB, TC], F32, tag="gmax")
        nc.vector.tensor_reduce(gmax[:], p3[:, :, 0:NBLK:8],
                                axis=mybir.AxisListType.X, op=ALU.max)
        nc.vector.tensor_scalar_mul(gmax[:], gmax[:], float(np.exp(-RHAT)))
        ginv32 = scratch.tile([PB, TC], F32, tag="ginv32")
        nc.vector.reciprocal(ginv32[:], gmax[:])
        nc.vector.tensor_copy(ginvb[h][:], ginv32[:])
        nc.sync.dma_start(ginv_out[:, h * TC:(h + 1) * TC], ginvb[h][:])
        for blk in range(NBLK):
            sl = ptil[h][:, blk * TC:(blk + 1) * TC]
            nc.gpsimd.tensor_mul(sl, sl, ginvb[h][:])

    def sb(s):  # Treg slot base col
        return (s + 2) * W

    def phase_d(k):
        t0 = k * TC
        if k > 0:
            start = 2 * W + t0
            bcols = treg_t[:, start:start + (S - 1) * W + 1:W]
            nc.vector.tensor_reduce(raw[:, k:k + 1], bcols,
                                    axis=mybir.AxisListType.X, op=ALU.max)
            rinv = scratch.tile([PB, 1], F32, tag="rinv")
            nc.vector.reciprocal(rinv[:], raw[:, k:k + 1])
            nc.vector.tensor_scalar_mul(bcols, bcols, rinv[:])
        for s in range(S):
            base = sb(s)
            if s % 2 == 1:
                j = (s - 1) // 2
                c = cpool.tile([PB, TC], F32, tag="c")
                nc.vector.scalar_tensor_tensor(
                    c[:],
                    treg_t[:, sb(s - 2) + t0: sb(s - 2) + t0 + TC],
                    m01[:, j:j + 1],
                    treg_t[:, sb(s - 1) + t0: sb(s - 1) + t0 + TC],
                    op0=ALU.mult, op1=ALU.add,
                )
                d0 = c[:]
                blk = j
            else:
                d0 = treg_t[:, sb(s - 1) + t0: sb(s - 1) + t0 + TC]
                blk = L
            # chunk 0: immediate initial (col-0 cells stay 0 — they feed the
            # t=0 coupling reads of rows s+1, s+2)
            if k == 0:
                initial = 1.0 if s <= 1 else 0.0
            else:
                initial = treg_t[:, base + t0: base + t0 + 1]
            nc.vector.tensor_tensor_scan(
                treg_t[:, base + 1 + t0: base + 1 + t0 + TC],
                d0,
                ptil[k][:, blk * TC:(blk + 1) * TC],
                initial,
                op0=ALU.add, op1=ALU.mult,
            )

    for h in range(NCH):
        phase_abc(h)
    for k in range(NCH):
        phase_d(k)

    b127 = sb(127) + T
    b128 = sb(128) + T
    nc.vector.tensor_add(raw[:, 0:1], treg_t[:, b127:b127 + 1],
                         treg_t[:, b128:b128 + 1])
    nc.sync.dma_start(raw_out[:], raw[:])


_CACHE: dict = {}


def _build():
    nc = bacc.Bacc("TRN2", target_bir_lowering=False, debug=False,
                   num_devices=NCORES, num_swdge_queues=4)
    y_in = nc.dram_tensor("ypred", [PB, T, C], F32, kind="ExternalInput").ap()
    idxs_in = nc.dram_tensor("idxs", [PB, NBLK * PB // 16], I16,
                             kind="ExternalInput").ap()
    m01_in = nc.dram_tensor("m01", [PB, L], F32, kind="ExternalInput").ap()
    raw_out = nc.dram_tensor("raw", [PB, NCH], F32, kind="ExternalOutput").ap()
    ginv_out = nc.dram_tensor("ginv", [PB, T], BF16, kind="ExternalOutput").ap()
    with tile.TileContext(nc) as tcx:
        with ExitStack() as ctx:
            _emit(ctx, tcx, y_in, idxs_in, m01_in, raw_out, ginv_out)
    nc.compile()
    return nc


def _run(in_maps, **kwargs):
    if "nc" not in _CACHE:
        _CACHE["nc"] = _build()
    return run_bass_kernel_spmd(_CACHE["nc"], in_maps,
                                core_ids=list(range(NCORES)), **kwargs)


def kernel(y_true: np.ndarray, y_pred: np.ndarray, **run_kwargs) -> np.ndarray:
    assert y_pred.shape == (B, T, C), y_pred.shape
    in_maps = []
    for c in range(NCORES):
        sl = slice(c * PB, (c + 1) * PB)
        prep = _host_prep(y_true[sl])
        in_maps.append({"ypred": np.ascontiguousarray(y_pred[sl], np.float32),
                        "idxs": prep["idxs"], "m01": prep["m01"]})
    res = _run(in_maps, **run_kwargs)
    raw = np.concatenate([res.results[c]["raw"] for c in range(NCORES)], axis=0)
    ginv = np.concatenate([res.results[c]["ginv"] for c in range(NCORES)],
                          axis=0).astype(np.float64)
    lng = np.log(ginv).sum(axis=1)
    val = np.log(raw[:, 0].astype(np.float64))
    val += np.log(raw[:, 1:].astype(np.float64)).sum(axis=1)
    loss = -(val - lng)
    if run_kwargs:
        kernel.last_results = res  # expose trace info to test harness
    return loss[:, None].astype(np.float32)
